# revision 1
# baseline (speedup 1.0000x reference)
"""Trainium2 Bass kernel for CLIP-style symmetric contrastive loss.

Problem: image_features [8192, 1024] f32, text_features [8192, 1024] f32.
  loss = 0.5 * (CE(logits, diag) + CE(logits.T, diag)),
  logits = cosine_similarity(img, txt) / 0.07.

Distribution: shard image rows across 8 NeuronCores. Each core m computes the
slab S_m = img_n[m] @ txt_n.T / T  ([1024, 8192]) against the full normalized
text matrix, reduces exp(S - C) along rows (local log-sum-exp) and along
columns (partial column sums), and a single [8194]-float AllReduce combines
the column sums plus the per-core scalar partials. Every core then finishes
the scalar loss locally.

The text matrix ships to the device pre-transposed ([D, N], bf16) so the
contraction dim lands on SBUF partitions with plain contiguous DMAs; its
normalization happens on-device in that layout (ACT squares + PE ones-matmul
partition reduction + per-chunk rsqrt scaling).

Math (C = 1/T upper-bounds every logit, so exp(S - C) <= 1 is stable):
  loss = C + (R + L - (2/T) * Draw) / (2N)
    R    = sum_i log sum_j exp(S_ij - C)
    L    = sum_j log sum_i exp(S_ij - C)
    Draw = sum_i cos(img_i, txt_i)
"""
import threading
from contextlib import ExitStack

import ml_dtypes
import numpy as np

import concourse.bacc as bacc
import concourse.bass as bass
import concourse.bass_isa as bass_isa
import concourse.mybir as mybir
import concourse.tile as tile
from concourse.bass_utils import run_bass_kernel_spmd

F32 = mybir.dt.float32
BF16 = mybir.dt.bfloat16
AF = mybir.ActivationFunctionType
ALU = mybir.AluOpType

N_CORES = 8
N = 8192
D = 1024
TEMPERATURE = 0.07


def build_nc(n=N, d=D, n_cores=N_CORES, no_collective=False, prep_only=False):
    """Build the SPMD Bass program (same program on every core)."""
    inv_t = float(1.0 / TEMPERATURE)
    cexp = float(1.0 / TEMPERATURE)          # stabilizer: max possible logit
    rows = n // n_cores                      # image rows per core
    P = 128
    rp = rows // P                           # row-tiles per core (8)
    kt = d // P                              # contraction tiles (8)
    CH = 512                                 # matmul free-dim chunk
    n_ch = n // CH                           # column chunks (16)
    cb_sz = min(4, n_ch)                     # chunks per psum block

    nc = bacc.Bacc("TRN2", target_bir_lowering=False, debug=False,
                   num_devices=n_cores)
    img = nc.dram_tensor("img", [rows, d], F32, kind="ExternalInput").ap()
    txt_t = nc.dram_tensor("txt_t", [d, n], BF16, kind="ExternalInput").ap()
    txt_own = nc.dram_tensor("txt_own", [rows, d], F32, kind="ExternalInput").ap()
    ones = nc.dram_tensor("ones", [P, P], F32, kind="ExternalInput").ap()
    ones_b = nc.dram_tensor("ones_b", [P, P], BF16, kind="ExternalInput").ap()
    ident = nc.dram_tensor("ident", [P, P], BF16, kind="ExternalInput").ap()
    out = nc.dram_tensor("out", [1, 1], F32, kind="ExternalOutput").ap()

    with tile.TileContext(nc) as tc:
        _body(tc, img, txt_t, txt_own, ones, ones_b, ident, out,
              n=n, d=d, rows=rows, P=P, rp=rp, kt=kt, CH=CH,
              n_ch=n_ch, cb_sz=cb_sz, inv_t=inv_t, cexp=cexp, n_cores=n_cores,
              no_collective=no_collective, prep_only=prep_only)
    nc.compile()
    return nc


def _body(tc, img, txt_t, txt_own, ones, ones_b, ident, out, *, n, d, rows, P,
          rp, kt, CH, n_ch, cb_sz, inv_t, cexp, n_cores, no_collective,
          prep_only):
    nc = tc.nc
    with ExitStack() as ctx:
        persist = ctx.enter_context(tc.tile_pool(name="persist", bufs=1))
        stage_f = ctx.enter_context(tc.tile_pool(name="stage_f", bufs=2))
        stage_b = ctx.enter_context(tc.tile_pool(name="stage_b", bufs=2))
        sqp = ctx.enter_context(tc.tile_pool(name="sqp", bufs=2))
        rbp = ctx.enter_context(tc.tile_pool(name="rbp", bufs=1))
        exp_p = ctx.enter_context(tc.tile_pool(name="exp_p", bufs=4))
        v1 = ctx.enter_context(tc.tile_pool(name="v1", bufs=6))
        csb_p = ctx.enter_context(tc.tile_pool(name="csb_p", bufs=1))
        rpp = ctx.enter_context(tc.tile_pool(name="rpp", bufs=2))
        psum = ctx.enter_context(tc.tile_pool(name="psum", bufs=4, space="PSUM"))
        ssq_ps = ctx.enter_context(tc.tile_pool(name="ssq_ps", bufs=2, space="PSUM"))
        tp_ps = ctx.enter_context(tc.tile_pool(name="tp_ps", bufs=2, space="PSUM"))
        dram = ctx.enter_context(tc.tile_pool(name="dram", bufs=1, space="DRAM"))

        txtT = persist.tile([P, kt, n], BF16, tag="txtT")       # [d-part, k, j]
        imgT = persist.tile([P, kt, rows], BF16, tag="imgT")    # [d-part, k, i]
        acc = persist.tile([P, n], F32, tag="acc")              # col partial sums
        vecs = persist.tile([P, 64], F32, tag="vecs")
        ones_sb = persist.tile([P, P], F32, tag="ones")
        ones_bsb = persist.tile([P, P], BF16, tag="ones_bsb")
        ident_sb = persist.tile([P, P], BF16, tag="ident")
        cs_sb = persist.tile([P, n // P], F32, tag="cs_sb")
        ln_cs = persist.tile([P, n // P], F32, tag="ln_cs")
        ebias = persist.tile([P, 1], F32, tag="ebias")

        cbuf = dram.tile([1, n + 64], F32, tag="cbuf")
        cbuf_out = dram.tile([1, n + 64], F32, tag="cbuf_out", addr_space="Shared")

        nc.sync.dma_start(ones_sb[:], ones[:])
        nc.sync.dma_start(ones_bsb[:], ones_b[:])
        nc.sync.dma_start(ident_sb[:], ident[:])
        nc.gpsimd.memset(ebias[:], float(-cexp))

        # vecs column map:
        RS = 0          # cols 0..rp-1   : per-row-tile rowsum(exp)
        DG = 8          # cols 8..8+rp-1 : per-row-tile diag cosine partials
        LNR = 16        # cols 16..: ln of rowsums
        SC = 56         # col 56: R partial, 57: Draw partial

        # --- Phase A: image prep (+ diag dot with own text rows) ------------
        for t in range(rp):
            img_raw = stage_f.tile([P, d], F32, tag="stage")
            nc.sync.dma_start(img_raw[:], img[t * P:(t + 1) * P, :])
            to_raw = stage_f.tile([P, d], F32, tag="stage")
            nc.sync.dma_start(to_raw[:], txt_own[t * P:(t + 1) * P, :])

            v = v1.tile([P, 8], F32, tag="v1")
            sq = stage_b.tile([P, d], BF16, tag="sq")
            nc.scalar.activation(sq[:], img_raw[:], AF.Square,
                                 accum_out=v[:, 0:1])
            nc.scalar.activation(v[:, 1:2], v[:, 0:1], AF.Sqrt)
            nc.vector.reciprocal(v[:, 2:3], v[:, 1:2])       # 1/||img_i||
            sq2 = stage_b.tile([P, d], BF16, tag="sq")
            nc.scalar.activation(sq2[:], to_raw[:], AF.Square,
                                 accum_out=v[:, 3:4])
            nc.scalar.activation(v[:, 4:5], v[:, 3:4], AF.Sqrt)
            nc.vector.reciprocal(v[:, 5:6], v[:, 4:5])       # 1/||txt_own_i||

            dot_scr = stage_b.tile([P, d], BF16, tag="sq")
            nc.vector.tensor_tensor(dot_scr[:], img_raw[:], to_raw[:], ALU.mult)
            nc.vector.tensor_reduce(v[:, 6:7], dot_scr[:],
                                    axis=mybir.AxisListType.X, op=ALU.add)
            nc.vector.tensor_tensor(v[:, 7:8], v[:, 2:3], v[:, 5:6], ALU.mult)
            nc.vector.tensor_tensor(vecs[:, DG + t:DG + t + 1], v[:, 6:7],
                                    v[:, 7:8], ALU.mult)     # diag cosine

            imgn_b = stage_b.tile([P, d], BF16, tag="nrm")
            nc.vector.tensor_scalar_mul(imgn_b[:], img_raw[:], v[:, 2:3])
            # transpose imgn_b [128 i, 1024 d] into imgT k-tiles via PE
            for k in range(kt):
                tp = tp_ps.tile([P, P], BF16, tag="tp")
                nc.tensor.transpose(tp[:], imgn_b[:, k * P:(k + 1) * P],
                                    ident_sb[:])
                nc.vector.tensor_copy(imgT[:, k, t * P:(t + 1) * P], tp[:])

        # --- Phase B: text load (pre-transposed bf16) + normalize in place --
        for k in range(kt):
            nc.sync.dma_start(txtT[:, k, :], txt_t[k * P:(k + 1) * P, :])
        for c in range(n_ch):
            sl = slice(c * CH, (c + 1) * CH)
            ssq = ssq_ps.tile([P, CH], F32, tag="ssq")
            for k in range(kt):
                sqc = sqp.tile([P, CH], BF16, tag="sqc")
                nc.scalar.activation(sqc[:], txtT[:, k, sl], AF.Square)
                nc.tensor.matmul(ssq[:], ones_bsb[:], sqc[:],
                                 start=(k == 0), stop=(k == kt - 1))
            nrm = rbp.tile([P, CH], BF16, tag="nrm_c")
            nc.scalar.activation(nrm[:], ssq[:], AF.Sqrt)
            rcp = rbp.tile([P, CH], F32, tag="rcp")
            nc.vector.reciprocal(rcp[:], nrm[:])
            rb = rbp.tile([P, CH], BF16, tag="rb")
            nc.vector.tensor_copy(rb[:], rcp[:])
            for k in range(kt):
                nc.vector.tensor_tensor(txtT[:, k, sl], txtT[:, k, sl],
                                        rb[:], ALU.mult)

        if prep_only:
            nc.vector.tensor_reduce(vecs[:, 30:31], txtT[:, 0, 0:CH],
                                    axis=mybir.AxisListType.X, op=ALU.add)
            nc.vector.tensor_reduce(vecs[:, 31:32], imgT[:, 0, 0:CH],
                                    axis=mybir.AxisListType.X, op=ALU.add)
            nc.sync.dma_start(out[0:1, 0:1], vecs[0:1, 30:31])
            return

        # --- Phase C: main matmul + exp + row/col reductions ----------------
        for p in range(rp):
            rparts = rpp.tile([P, n_ch], F32, tag="rp")
            for cb in range(n_ch // cb_sz):
                mms = []
                for _ci in range(cb_sz):
                    mm_t = psum.tile([P, CH], F32, tag="mm")
                    mms.append(mm_t)
                for k in range(kt):
                    for ci in range(cb_sz):
                        c = cb * cb_sz + ci
                        nc.tensor.matmul(
                            mms[ci][:],
                            imgT[:, k, p * P:(p + 1) * P],
                            txtT[:, k, c * CH:(c + 1) * CH],
                            start=(k == 0), stop=(k == kt - 1))
                for ci in range(cb_sz):
                    c = cb * cb_sz + ci
                    ex = exp_p.tile([P, CH], BF16, tag="exp")
                    nc.scalar.activation(ex[:], mms[ci][:], AF.Exp,
                                         bias=ebias[:, 0:1], scale=inv_t,
                                         accum_out=rparts[:, c:c + 1])
                    sl = slice(c * CH, (c + 1) * CH)
                    if p == 0:
                        nc.vector.tensor_copy(acc[:, sl], ex[:])
                    else:
                        nc.vector.tensor_tensor(acc[:, sl], acc[:, sl], ex[:],
                                                ALU.add)
            nc.vector.tensor_reduce(vecs[:, RS + p:RS + p + 1], rparts[:],
                                    axis=mybir.AxisListType.X, op=ALU.add)

        # --- Phase D: local scalars -----------------------------------------
        nc.scalar.activation(vecs[:, LNR:LNR + rp], vecs[:, RS:RS + rp], AF.Ln)
        nc.vector.tensor_reduce(vecs[:, 24:25], vecs[:, LNR:LNR + rp],
                                axis=mybir.AxisListType.X, op=ALU.add)
        nc.gpsimd.partition_all_reduce(vecs[:, SC:SC + 1], vecs[:, 24:25],
                                       channels=P, reduce_op=bass_isa.ReduceOp.add)
        nc.vector.tensor_reduce(vecs[:, 25:26], vecs[:, DG:DG + rp],
                                axis=mybir.AxisListType.X, op=ALU.add)
        nc.gpsimd.partition_all_reduce(vecs[:, SC + 1:SC + 2], vecs[:, 25:26],
                                       channels=P, reduce_op=bass_isa.ReduceOp.add)

        # column partial sums (reduce acc over partitions via ones-matmul)
        for c in range(n_ch):
            ps = psum.tile([P, CH], F32, tag="mm")
            nc.tensor.matmul(ps[:], ones_sb[:], acc[:, c * CH:(c + 1) * CH],
                             start=True, stop=True)
            csb = csb_p.tile([P, CH], F32, tag="csb")
            nc.vector.tensor_copy(csb[0:1, :], ps[0:1, :])
            nc.sync.dma_start(cbuf[0:1, c * CH:(c + 1) * CH], csb[0:1, :])
        nc.sync.dma_start(cbuf[0:1, n:n + 2], vecs[0:1, SC:SC + 2])

        # --- Phase E: AllReduce + finish -------------------------------------
        if no_collective:
            nc.sync.dma_start(cbuf_out[:], cbuf[:])
        else:
            nc.gpsimd.collective_compute(
                "AllReduce", ALU.add,
                replica_groups=[list(range(n_cores))],
                ins=[cbuf[:].opt()], outs=[cbuf_out[:].opt()])

        nc.sync.dma_start(
            cs_sb[:], cbuf_out[0:1, 0:n].rearrange("a (p x) -> (a p) x", p=P))
        nc.scalar.activation(ln_cs[:], cs_sb[:], AF.Ln)
        nc.vector.tensor_reduce(vecs[:, 26:27], ln_cs[:],
                                axis=mybir.AxisListType.X, op=ALU.add)
        nc.gpsimd.partition_all_reduce(vecs[:, 27:28], vecs[:, 26:27],
                                       channels=P, reduce_op=bass_isa.ReduceOp.add)
        rd = v1.tile([P, 8], F32, tag="v1")
        nc.sync.dma_start(rd[0:1, 0:2], cbuf_out[0:1, n:n + 2])

        # loss = cexp + (R + L - (2/T) * Draw) / (2N)
        fin = v1.tile([P, 8], F32, tag="v1")
        nc.vector.tensor_tensor(fin[0:1, 0:1], rd[0:1, 0:1],
                                vecs[0:1, 27:28], ALU.add)          # R + L
        nc.vector.tensor_scalar_mul(fin[0:1, 1:2], rd[0:1, 1:2],
                                    float(-2.0 * inv_t))            # -(2/T) Draw
        nc.vector.tensor_tensor(fin[0:1, 2:3], fin[0:1, 0:1],
                                fin[0:1, 1:2], ALU.add)
        nc.scalar.activation(fin[0:1, 3:4], fin[0:1, 2:3], AF.Copy,
                             bias=float(cexp), scale=float(1.0 / (2 * n)))
        nc.sync.dma_start(out[0:1, 0:1], fin[0:1, 3:4])


def make_in_maps(image_features, text_features, n=N, d=D, n_cores=N_CORES):
    image_features = np.asarray(image_features, dtype=np.float32)
    text_features = np.asarray(text_features, dtype=np.float32)
    rows = n // n_cores
    txt_t = np.ascontiguousarray(text_features.T).astype(ml_dtypes.bfloat16)
    ones = np.ones((128, 128), dtype=np.float32)
    ones_b = np.ones((128, 128), dtype=ml_dtypes.bfloat16)
    ident = np.eye(128, dtype=np.float32).astype(ml_dtypes.bfloat16)
    return [
        {
            "img": image_features[m * rows:(m + 1) * rows],
            "txt_t": txt_t,
            "txt_own": text_features[m * rows:(m + 1) * rows],
            "ones": ones,
            "ones_b": ones_b,
            "ident": ident,
        }
        for m in range(n_cores)
    ]


_CACHE = {}
_LOCK = threading.Lock()


def _get_nc():
    with _LOCK:
        if "nc" not in _CACHE:
            _CACHE["nc"] = build_nc()
        return _CACHE["nc"]


def kernel(image_features, text_features):
    image_features = np.asarray(image_features, dtype=np.float32)
    text_features = np.asarray(text_features, dtype=np.float32)
    assert image_features.shape == (N, D) and text_features.shape == (N, D)
    nc = _get_nc()
    in_maps = make_in_maps(image_features, text_features)
    res = run_bass_kernel_spmd(nc, in_maps, list(range(N_CORES)))
    val = np.float32(res.results[0]["out"][0, 0])
    return np.array(val, dtype=np.float32)



# revision 2
# speedup vs baseline: 1.3886x; 1.3886x over previous
"""Trainium2 Bass kernel for CLIP-style symmetric contrastive loss.

Problem: image_features [8192, 1024] f32, text_features [8192, 1024] f32.
  loss = 0.5 * (CE(logits, diag) + CE(logits.T, diag)),
  logits = cosine_similarity(img, txt) / 0.07.

Distribution: shard image rows (and text rows, for norm/diag work) across 8
cores. Each core computes the raw-dot slab S'_m = q(img_m) @ q(txt).T in fp8
DoubleRow mode ([1024, 8192] per core), then applies the cosine normalization
AFTER the GEMM: per-column 1/||txt_j|| on DVE (broadcast row from an early
AllGather of per-core own-text norms), per-row 1/||img_i|| folded into the
ACT exp's per-partition scale. Row sums come free via ACT accum_out; column
partial sums accumulate in bf16 on DVE and are partition-reduced with M=1
ones-matmuls. Two staged AllReduces combine column sums + scalar partials.

Math (C = 1/T bounds every logit since cosines of quantized vectors <= ~1):
  loss = C + (R + L - (2/T) * Draw) / (2N)
    R    = sum_i log sum_j exp(s_ij/T - C)
    L    = sum_j log sum_i exp(s_ij/T - C)
    Draw = sum_i cos(img_i, txt_i)
"""
import threading
from contextlib import ExitStack

import ml_dtypes
import numpy as np

import concourse.bacc as bacc
import concourse.bass_isa as bass_isa
import concourse.mybir as mybir
import concourse.tile as tile
from concourse.bass_utils import run_bass_kernel_spmd

F32 = mybir.dt.float32
BF16 = mybir.dt.bfloat16
FP8 = mybir.dt.float8e4
AF = mybir.ActivationFunctionType
ALU = mybir.AluOpType

N_CORES = 8
N = 8192
D = 1024
TEMPERATURE = 0.07

USE_DR = True        # fp8 DoubleRow main GEMM (else bf16)
GP_BCAST = True      # gpsimd partition_broadcast for the txt-norm row


def build_nc(n=N, d=D, n_cores=N_CORES, use_dr=USE_DR):
    inv_t = float(1.0 / TEMPERATURE)
    cexp = float(1.0 / TEMPERATURE)
    rows = n // n_cores                  # img/txt rows per core (1024)
    P = 128
    kt = d // P                          # 128-row contraction tiles (8)
    CH = 512                             # matmul free-dim chunk
    CB = 1024                            # column block (2 chunks)
    n_cb = n // CB                       # 8 column blocks
    rp = rows // P                       # row-tiles per core (8)
    in_dt = FP8 if use_dr else BF16

    nc = bacc.Bacc("TRN2", target_bir_lowering=False, debug=False,
                   num_devices=n_cores)
    imgT = nc.dram_tensor("imgT", [d, rows], in_dt, kind="ExternalInput").ap()
    txtT = nc.dram_tensor("txtT", [d, n], in_dt, kind="ExternalInput").ap()
    txtoT = nc.dram_tensor("txtoT", [d, rows], in_dt, kind="ExternalInput").ap()
    ones1 = nc.dram_tensor("ones1", [P, 1], BF16, kind="ExternalInput").ap()
    onesr = nc.dram_tensor("onesr", [1, P], BF16, kind="ExternalInput").ap()
    out = nc.dram_tensor("out", [1, 1], F32, kind="ExternalOutput").ap()

    with tile.TileContext(nc) as tc:
        _body(tc, imgT, txtT, txtoT, ones1, onesr, out,
              n=n, d=d, rows=rows, P=P, kt=kt, CH=CH, CB=CB, n_cb=n_cb,
              rp=rp, inv_t=inv_t, cexp=cexp, n_cores=n_cores, use_dr=use_dr,
              in_dt=in_dt)
    nc.compile()
    return nc


def _body(tc, imgT, txtT, txtoT, ones1, onesr, out, *, n, d, rows, P, kt, CH,
          CB, n_cb, rp, inv_t, cexp, n_cores, use_dr, in_dt):
    nc = tc.nc
    half = n // 2
    with ExitStack() as ctx:
        persist = ctx.enter_context(tc.tile_pool(name="persist", bufs=1))
        spool = ctx.enter_context(tc.tile_pool(name="spool", bufs=3))
        epool = ctx.enter_context(tc.tile_pool(name="epool", bufs=3))
        rpool = ctx.enter_context(tc.tile_pool(name="rpool", bufs=2))
        stg = ctx.enter_context(tc.tile_pool(name="stg", bufs=2))
        sq0p = ctx.enter_context(tc.tile_pool(name="sq0p", bufs=3))
        mmp = ctx.enter_context(tc.tile_pool(name="mmp", bufs=3, space="PSUM"))
        aux = ctx.enter_context(tc.tile_pool(name="aux", bufs=2, space="PSUM"))
        dram = ctx.enter_context(tc.tile_pool(name="dram", bufs=1, space="DRAM"))

        txt_sb = persist.tile([P, kt, n], in_dt, tag="txt_sb")
        img_sb = persist.tile([P, kt, rows], in_dt, tag="img_sb")
        txto_sb = persist.tile([P, kt, rows], in_dt, tag="txto_sb")
        sqb = persist.tile([P, kt, rows], BF16, tag="sqb")
        dpb = persist.tile([P, kt, rows], BF16, tag="dpb")
        rbcast = persist.tile([P, n], BF16, tag="rbcast")
        acc = persist.tile([P, n], BF16, tag="acc")
        rowp = persist.tile([P, 64], F32, tag="rowp")
        rimg_sc = persist.tile([P, kt], F32, tag="rimg_sc")
        ones1_sb = persist.tile([P, 1], BF16, tag="ones1")
        onesr_sb = persist.tile([1, P], BF16, tag="onesr")
        ebias = persist.tile([P, 1], F32, tag="ebias")
        rows_sb = persist.tile([1, 4 * rows + 64], F32, tag="rows_sb")
        # regions: rtxt f32, rimg f32, dcos scratch, rimg*inv_t, scalars
        R_RTX, R_RIM, R_DC, R_RIS = 0, rows, 2 * rows, 3 * rows
        SC = 4 * rows
        vec = persist.tile([P, 16], F32, tag="vec")

        ag_in = dram.tile([1, rows], BF16, tag="ag_in")
        ag_out = dram.tile([1, n], BF16, tag="ag_out", addr_space="Shared")
        cbA = dram.tile([1, half], F32, tag="cbA")
        cbA_out = dram.tile([1, half], F32, tag="cbA_out", addr_space="Shared")
        cbB = dram.tile([1, half + 16], F32, tag="cbB")
        cbB_out = dram.tile([1, half + 16], F32, tag="cbB_out",
                            addr_space="Shared")
        rr_dram = dram.tile([1, rows], F32, tag="rr_dram")

        nc.sync.dma_start(ones1_sb[:], ones1[:])
        nc.sync.dma_start(onesr_sb[:], onesr[:])
        nc.gpsimd.memset(ebias[:], float(-cexp))

        # Pin the natural_log_exp table set with a tiny Ln first.
        nc.scalar.activation(vec[0:1, 15:16], ones1_sb[0:1, 0:1], AF.Ln)

        # --- bulk input DMAs: 4 consolidated triggers ----------------------
        nc.sync.dma_start(
            txt_sb[:, :, 0:half],
            txtT[:, 0:half].rearrange("(k p) x -> p k x", p=P))
        nc.sync.dma_start(txto_sb[:],
                          txtoT[:, :].rearrange("(k p) x -> p k x", p=P))
        nc.sync.dma_start(img_sb[:],
                          imgT[:, :].rearrange("(k p) x -> p k x", p=P))
        nc.sync.dma_start(
            txt_sb[:, :, half:n],
            txtT[:, half:n].rearrange("(k p) x -> p k x", p=P))

        lnscr = persist.tile([1, rows], F32, tag="lnscr")
        r0b = persist.tile([1, CB], BF16, tag="r0b")
        rtxt_b = persist.tile([1, rows], BF16, tag="rtxt_b")

        # --- local txt-norms for column block 0 (ACT squares) ---------------
        ps0 = aux.tile([1, CH], F32, tag="aps")
        ps1 = aux.tile([1, CH], F32, tag="aps")
        for k in range(kt):
            sq = sq0p.tile([P, CB], BF16, tag="sq0")
            nc.scalar.activation(sq[:], txt_sb[:, k, 0:CB], AF.Square)
            nc.tensor.matmul(ps0[:], ones1_sb[:], sq[:, 0:CH],
                             start=(k == 0), stop=(k == kt - 1))
            nc.tensor.matmul(ps1[:], ones1_sb[:], sq[:, CH:CB],
                             start=(k == 0), stop=(k == kt - 1))
        nc.scalar.activation(lnscr[0:1, 0:CH], ps0[:], AF.Ln)
        nc.scalar.activation(lnscr[0:1, CH:CB], ps1[:], AF.Ln)
        nc.scalar.activation(r0b[:], lnscr[0:1, 0:CB], AF.Exp, scale=-0.5)

        # --- own-text norms (DVE squares) -> AllGather ----------------------
        for k in range(kt):
            nc.vector.tensor_tensor(sqb[:, k, :], txto_sb[:, k, :],
                                    txto_sb[:, k, :], ALU.mult)
        for h in range(2):
            ps = aux.tile([1, CH], F32, tag="aps")
            for k in range(kt):
                nc.tensor.matmul(ps[:], ones1_sb[:],
                                 sqb[:, k, h * CH:(h + 1) * CH],
                                 start=(k == 0), stop=(k == kt - 1))
            nc.scalar.activation(lnscr[0:1, h * CH:(h + 1) * CH], ps[:],
                                 AF.Ln)
        nc.scalar.activation(rows_sb[0:1, R_RTX:R_RTX + rows],
                             lnscr[0:1, 0:rows], AF.Exp, scale=-0.5)
        nc.vector.tensor_copy(rtxt_b[:], rows_sb[0:1, R_RTX:R_RTX + rows])

        # --- img norms (ACT squares, sqb reused after txto MMs) -------------
        for k in range(kt):
            nc.scalar.activation(sqb[:, k, :], img_sb[:, k, :], AF.Square)
        for h in range(2):
            ps = aux.tile([1, CH], F32, tag="aps")
            for k in range(kt):
                nc.tensor.matmul(ps[:], ones1_sb[:],
                                 sqb[:, k, h * CH:(h + 1) * CH],
                                 start=(k == 0), stop=(k == kt - 1))
            nc.scalar.activation(lnscr[0:1, h * CH:(h + 1) * CH], ps[:],
                                 AF.Ln)
        nc.scalar.activation(rows_sb[0:1, R_RIM:R_RIM + rows],
                             lnscr[0:1, 0:rows], AF.Exp, scale=-0.5)
        nc.vector.tensor_scalar_mul(rows_sb[0:1, R_RIS:R_RIS + rows],
                                    rows_sb[0:1, R_RIM:R_RIM + rows],
                                    float(inv_t))
        nc.sync.dma_start(rr_dram[:], rows_sb[0:1, R_RIS:R_RIS + rows])
        nc.sync.dma_start(
            rimg_sc[:], rr_dram[0:1, :].rearrange("a (x p) -> (a p) x", p=P))

        # --- gpsimd queue: broadcasts + AllGather (order matters: FIFO) -----
        nc.gpsimd.partition_broadcast(rbcast[:, 0:CB], r0b[0:1, :],
                                      channels=P)
        nc.gpsimd.dma_start(ag_in[:], rtxt_b[:])
        nc.gpsimd.collective_compute(
            "AllGather", ALU.bypass,
            replica_groups=[list(range(n_cores))],
            ins=[ag_in[:].opt()], outs=[ag_out[:].opt()])
        rrow = persist.tile([1, n], BF16, tag="rrow")
        nc.gpsimd.dma_start(rrow[0:1, CB:2 * CB], ag_out[0:1, CB:2 * CB])
        nc.gpsimd.partition_broadcast(rbcast[:, CB:2 * CB],
                                      rrow[0:1, CB:2 * CB], channels=P)
        nc.gpsimd.dma_start(rrow[0:1, 2 * CB:n], ag_out[0:1, 2 * CB:n])
        nc.gpsimd.partition_broadcast(rbcast[:, 2 * CB:n],
                                      rrow[0:1, 2 * CB:n], channels=P)

        # --- diag cosines (Draw partial) ------------------------------------
        for k in range(kt):
            nc.vector.tensor_tensor(dpb[:, k, :], img_sb[:, k, :],
                                    txto_sb[:, k, :], ALU.mult)
        for h in range(2):
            ps = aux.tile([1, CH], F32, tag="aps")
            for k in range(kt):
                nc.tensor.matmul(ps[:], ones1_sb[:],
                                 dpb[:, k, h * CH:(h + 1) * CH],
                                 start=(k == 0), stop=(k == kt - 1))
            sl = slice(R_DC + h * CH, R_DC + (h + 1) * CH)
            nc.vector.tensor_tensor(rows_sb[0:1, sl], ps[:],
                                    rows_sb[0:1, R_RIM + h * CH:
                                            R_RIM + (h + 1) * CH], ALU.mult)
        nc.vector.tensor_tensor(rows_sb[0:1, R_DC:R_DC + rows],
                                rows_sb[0:1, R_DC:R_DC + rows],
                                rows_sb[0:1, R_RTX:R_RTX + rows], ALU.mult)
        nc.vector.tensor_reduce(rows_sb[0:1, SC + 1:SC + 2],
                                rows_sb[0:1, R_DC:R_DC + rows],
                                axis=mybir.AxisListType.X, op=ALU.add)

        # --- main loop: GEMM -> scale -> exp -> row/col accumulation --------
        for cb in range(n_cb):
            cbs = slice(cb * CB, (cb + 1) * CB)
            for p in range(rp):
                mm = mmp.tile([P, CB], F32, tag="mm")
                if use_dr:
                    for g in range(kt // 2):
                        for ci in range(CB // CH):
                            c0 = cb * CB + ci * CH
                            nc.tensor.matmul(
                                mm[:, ci * CH:(ci + 1) * CH],
                                img_sb[:, 2 * g:2 * g + 2,
                                       p * P:(p + 1) * P],
                                txt_sb[:, 2 * g:2 * g + 2, c0:c0 + CH],
                                start=(g == 0), stop=(g == kt // 2 - 1),
                                perf_mode=mybir.MatmulPerfMode.DoubleRow)
                else:
                    for k in range(kt):
                        for ci in range(CB // CH):
                            c0 = cb * CB + ci * CH
                            nc.tensor.matmul(
                                mm[:, ci * CH:(ci + 1) * CH],
                                img_sb[:, k, p * P:(p + 1) * P],
                                txt_sb[:, k, c0:c0 + CH],
                                start=(k == 0), stop=(k == kt - 1))
                s = spool.tile([P, CB], BF16, tag="s")
                nc.vector.tensor_tensor(s[:], mm[:], rbcast[:, cbs], ALU.mult)
                e = epool.tile([P, CB], BF16, tag="e")
                nc.scalar.activation(e[:], s[:], AF.Exp,
                                     bias=ebias[:, 0:1],
                                     scale=rimg_sc[:, p:p + 1],
                                     accum_out=rowp[:, p * n_cb + cb:
                                                    p * n_cb + cb + 1])
                if p == 0:
                    nc.vector.tensor_copy(acc[:, cbs], e[:])
                else:
                    nc.vector.tensor_tensor(acc[:, cbs], acc[:, cbs], e[:],
                                            ALU.add)
            # column partial sums for this block -> DRAM
            for ci in range(CB // CH):
                c0 = cb * CB + ci * CH
                ps = aux.tile([1, CH], F32, tag="aps")
                nc.tensor.matmul(ps[:], ones1_sb[:], acc[:, c0:c0 + CH],
                                 start=True, stop=True)
                st = stg.tile([1, CH], F32, tag="st")
                nc.vector.tensor_copy(st[:], ps[:])
                if c0 < half:
                    nc.sync.dma_start(cbA[0:1, c0:c0 + CH], st[:])
                else:
                    nc.sync.dma_start(cbB[0:1, c0 - half:c0 - half + CH],
                                      st[:])
            if cb == n_cb // 2 - 1:
                nc.gpsimd.collective_compute(
                    "AllReduce", ALU.add,
                    replica_groups=[list(range(n_cores))],
                    ins=[cbA[:].opt()], outs=[cbA_out[:].opt()])

        # --- R partial (rowsums over all column blocks) ---------------------
        for p in range(rp):
            nc.vector.tensor_reduce(vec[:, p:p + 1],
                                    rowp[:, p * n_cb:(p + 1) * n_cb],
                                    axis=mybir.AxisListType.X, op=ALU.add)
        nc.scalar.activation(vec[:, 8:8 + rp], vec[:, 0:rp], AF.Ln)
        nc.vector.tensor_reduce(vec[:, 0:1], vec[:, 8:8 + rp],
                                axis=mybir.AxisListType.X, op=ALU.add)
        nc.gpsimd.partition_all_reduce(vec[:, 1:2], vec[:, 0:1],
                                       channels=P,
                                       reduce_op=bass_isa.ReduceOp.add)
        nc.vector.tensor_copy(rows_sb[0:1, SC:SC + 1], vec[0:1, 1:2])
        nc.sync.dma_start(cbB[0:1, half:half + 2],
                          rows_sb[0:1, SC:SC + 2])
        nc.gpsimd.collective_compute(
            "AllReduce", ALU.add,
            replica_groups=[list(range(n_cores))],
            ins=[cbB[:].opt()], outs=[cbB_out[:].opt()])

        # --- finish: L = sum_j ln(colsum_j), then the scalar loss -----------
        cs = persist.tile([P, n // P], F32, tag="cs")
        nc.sync.dma_start(
            cs[:, 0:half // P],
            cbA_out[0:1, :].rearrange("a (p x) -> (a p) x", p=P))
        nc.sync.dma_start(
            cs[:, half // P:n // P],
            cbB_out[0:1, 0:half].rearrange("a (p x) -> (a p) x", p=P))
        lncs = persist.tile([P, n // P], F32, tag="lncs")
        nc.scalar.activation(lncs[:], cs[:], AF.Ln)
        nc.vector.tensor_reduce(vec[:, 2:3], lncs[:],
                                axis=mybir.AxisListType.X, op=ALU.add)
        nc.gpsimd.partition_all_reduce(vec[:, 3:4], vec[:, 2:3],
                                       channels=P,
                                       reduce_op=bass_isa.ReduceOp.add)
        sc2 = persist.tile([1, 2], F32, tag="sc2")
        nc.sync.dma_start(sc2[:], cbB_out[0:1, half:half + 2])

        fin = persist.tile([1, 8], F32, tag="fin")
        nc.vector.tensor_tensor(fin[0:1, 0:1], sc2[0:1, 0:1],
                                vec[0:1, 3:4], ALU.add)            # R + L
        nc.vector.tensor_scalar_mul(fin[0:1, 1:2], sc2[0:1, 1:2],
                                    float(-2.0 * inv_t))           # -(2/T)*Draw
        nc.vector.tensor_tensor(fin[0:1, 2:3], fin[0:1, 0:1],
                                fin[0:1, 1:2], ALU.add)
        nc.scalar.activation(fin[0:1, 3:4], fin[0:1, 2:3], AF.Copy,
                             bias=float(cexp),
                             scale=float(1.0 / (2 * n)))
        nc.sync.dma_start(out[0:1, 0:1], fin[0:1, 3:4])


def make_in_maps(image_features, text_features, n=N, d=D, n_cores=N_CORES,
                 use_dr=USE_DR):
    image_features = np.asarray(image_features, dtype=np.float32)
    text_features = np.asarray(text_features, dtype=np.float32)
    np_dt = ml_dtypes.float8_e4m3 if use_dr else ml_dtypes.bfloat16
    rows = n // n_cores
    txtT = np.ascontiguousarray(text_features.T).astype(np_dt)
    ones1 = np.ones((128, 1), dtype=ml_dtypes.bfloat16)
    onesr = np.ones((1, 128), dtype=ml_dtypes.bfloat16)
    maps = []
    for m in range(n_cores):
        sl = slice(m * rows, (m + 1) * rows)
        maps.append({
            "imgT": np.ascontiguousarray(image_features[sl].T).astype(np_dt),
            "txtT": txtT,
            "txtoT": np.ascontiguousarray(text_features[sl].T).astype(np_dt),
            "ones1": ones1,
            "onesr": onesr,
        })
    return maps


_CACHE = {}
_LOCK = threading.Lock()


def _get_nc():
    with _LOCK:
        if "nc" not in _CACHE:
            _CACHE["nc"] = build_nc()
        return _CACHE["nc"]


def kernel(image_features, text_features):
    image_features = np.asarray(image_features, dtype=np.float32)
    text_features = np.asarray(text_features, dtype=np.float32)
    assert image_features.shape == (N, D) and text_features.shape == (N, D)
    nc = _get_nc()
    in_maps = make_in_maps(image_features, text_features)
    res = run_bass_kernel_spmd(nc, in_maps, list(range(N_CORES)))
    val = np.float32(res.results[0]["out"][0, 0])
    return np.array(val, dtype=np.float32)
